# revision 1
# baseline (speedup 1.0000x reference)
"""Multi-head attention Trainium2 kernel (B=4, N=2048, D=1024, H=16).

Sharding: 8 cores = 4 batches x 2 head-groups (8 heads each), zero
collectives. Each core:
  - projections in fp16 (k/v + first q chunk up front; later q chunks are
    projected mid-attention as PE gap filler)
  - q,k kept transposed [feat, seq]; v row-layout, augmented with a ones
    column so the PV matmul emits the softmax denominator for free (M=65)
  - attention per head-pair x 512-query chunk: S matmuls packed two heads
    per pass via disjoint PE row groups into one [128,1024] PSUM tile,
    one wide exp on ACT (scale=1/8, fp16 out), PV accumulation
  - normalization: denominator broadcast across partitions via a K=1
    ones-matmul on the raw denominator, then reciprocal+multiply on DVE
    (reciprocal runs on the broadcast result so the PE never waits on it);
    bcast+normalize emission lags two units behind compute
  - out-projection partial [1024,2048] per chunk, emission deferred past
    the next chunk's start so the PE queue never blocks on the DVE chain
Host sums the two head-group partials per batch and adds bias.
Measured: ~440us per core on trn2, absmax relative error ~7e-4.
"""
from contextlib import ExitStack

import numpy as np

import concourse.mybir as mybir
import concourse.tile as tile
from concourse import bacc
from concourse.bass_utils import run_bass_kernel_spmd

F32 = mybir.dt.float32
F32R = mybir.dt.float32r
F16 = mybir.dt.float16

P = 128
N = 2048         # sequence length
DI = 1024        # model dim
NH = 8           # heads per core
HD = 64          # head dim
NPAIR = 4        # head pairs per core
KT = 8           # contraction tiles for projections
CH = 512         # query chunk width
NCHUNK = 4       # chunks per sequence
MT = 16          # key tiles (m) per sequence
ET = 8           # output-feature blocks
SCALE = HD ** -0.5

_NC_CACHE = None


def _build():
    nc = bacc.Bacc("TRN2", target_bir_lowering=False, debug=False)

    xT = nc.dram_tensor("xT", [DI, N], F16, kind="ExternalInput").ap()
    wqkA = nc.dram_tensor("wqkA", [8, P, KT, P], F16, kind="ExternalInput").ap()
    wvA = nc.dram_tensor("wvA", [P, KT, 512], F16, kind="ExternalInput").ap()
    woT = nc.dram_tensor("woT", [512, DI], F16, kind="ExternalInput").ap()
    onesd = nc.dram_tensor("ones", [P, HD], F16, kind="ExternalInput").ap()
    outT = nc.dram_tensor("outT", [DI, N], F32, kind="ExternalOutput").ap()

    xT_r = xT.rearrange("(k p) n -> k p n", p=P)        # [8, 128, 2048]
    woT_r = woT.rearrange("(k p) e -> k p e", p=P)      # [4, 128, 1024]
    outT_r = outT.rearrange("(e p) n -> e p n", p=P)    # [8, 128, 2048]

    with tile.TileContext(nc) as tc, ExitStack() as persist:
        qk_pool = persist.enter_context(tc.tile_pool(name="qkp", bufs=8))
        va_pool = persist.enter_context(tc.tile_pool(name="vap", bufs=1))
        misc = persist.enter_context(tc.tile_pool(name="misc", bufs=1))

        ones_r = misc.tile([P, HD], F16)
        nc.sync.dma_start(ones_r[:], onesd[:])

        # qkT tiles: 0..3 = q head-pairs, 4..7 = k head-pairs.
        # Tile j holds heads 2j (parts 0:64) and 2j+1 (parts 64:128).
        qkT = [qk_pool.tile([P, N], F16, name=f"qkT{t}", tag="qkT")
               for t in range(8)]
        v_aug = va_pool.tile([P, MT, NH, HD + 1], F16)
        nc.vector.tensor_copy(v_aug[:, :, :, HD:HD + 1],
                              ones_r[:, 0:1].to_broadcast((P, MT, NH, 1)))

        xt_pool = persist.enter_context(tc.tile_pool(name="xt", bufs=8))
        wq_pool = persist.enter_context(tc.tile_pool(name="wq", bufs=4))
        xt = []
        for k in range(KT):
            t = xt_pool.tile([P, N], F16, name=f"xt{k}", tag="xt")
            nc.sync.dma_start(t[:], xT_r[k])
            xt.append(t)
        wq = []
        for f in range(4):
            t = wq_pool.tile([P, KT, P], F16, name=f"wq{f}", tag="wq")
            nc.sync.dma_start(t[:], wqkA[f])
            wq.append(t)

        # ---------------- Phase 1: k/v projections + first q chunk --------
        with ExitStack() as ph1:
            wqk_pool = ph1.enter_context(tc.tile_pool(name="wqk", bufs=2))
            wv_pool = ph1.enter_context(tc.tile_pool(name="wv", bufs=1))
            pp = ph1.enter_context(tc.tile_pool(name="pp", bufs=4, space="PSUM"))

            wv = wv_pool.tile([P, KT, 512], F16)
            nc.sync.dma_start(wv[:], wvA[:])

            # k projection: feature blocks 4..7, all chunks
            for f in range(4, 8):
                wqk_f = wqk_pool.tile([P, KT, P], F16, tag="wqk")
                nc.sync.dma_start(wqk_f[:], wqkA[f])
                for c in range(NCHUNK):
                    ps = pp.tile([P, CH], F32, tag="pp")
                    for k in range(KT):
                        nc.tensor.matmul(ps[:], wqk_f[:, k, :],
                                         xt[k][:, c * CH:(c + 1) * CH],
                                         start=(k == 0), stop=(k == KT - 1))
                    nc.vector.tensor_copy(qkT[f][:, c * CH:(c + 1) * CH], ps[:])

            # v projection: row block r
            for r in range(MT):
                ps = pp.tile([P, CH], F32, tag="pp")
                for k in range(KT):
                    nc.tensor.matmul(ps[:], xt[k][:, r * P:(r + 1) * P],
                                     wv[:, k, :],
                                     start=(k == 0), stop=(k == KT - 1))
                nc.vector.tensor_copy(v_aug[:, r, :, 0:HD],
                                      ps.rearrange("p (h d) -> p h d", d=HD))

            # q projection for chunk 0 only
            for f in range(4):
                ps = pp.tile([P, CH], F32, tag="pp")
                for k in range(KT):
                    nc.tensor.matmul(ps[:], wq[f][:, k, :], xt[k][:, 0:CH],
                                     start=(k == 0), stop=(k == KT - 1))
                nc.vector.tensor_copy(qkT[f][:, 0:CH], ps[:])

        # ---------------- Phase 2: attention + out-projection ----------------
        with ExitStack() as ph2:
            wo_pool = ph2.enter_context(tc.tile_pool(name="wo", bufs=4))
            exp_pool = ph2.enter_context(tc.tile_pool(name="expp", bufs=18))
            ot_pool = ph2.enter_context(tc.tile_pool(name="ot", bufs=8))
            osb_pool = ph2.enter_context(tc.tile_pool(name="osb", bufs=8))
            stage_pool = ph2.enter_context(tc.tile_pool(name="stg", bufs=3))
            den_pool = ph2.enter_context(tc.tile_pool(name="den", bufs=8))
            rbc_pool = ph2.enter_context(tc.tile_pool(name="rbc", bufs=4))
            sps_pool = ph2.enter_context(
                tc.tile_pool(name="sps", bufs=2, space="PSUM"))
            oaug_pool = ph2.enter_context(
                tc.tile_pool(name="oaug", bufs=2, space="PSUM"))
            aux_pool = ph2.enter_context(
                tc.tile_pool(name="aux", bufs=2, space="PSUM"))

            wo = [wo_pool.tile([P, DI], F16, name=f"wo{kk}", tag="wo")
                  for kk in range(NPAIR)]
            for kk in range(NPAIR):
                nc.sync.dma_start(wo[kk][:], woT_r[kk])

            ot_map = {}

            def emit_unit(c, p):
                csl = slice(c * CH, (c + 1) * CH)
                qA = qkT[p][0:HD, csl]
                qB = qkT[p][HD:P, csl]
                kTl = qkT[4 + p]

                oaugA = oaug_pool.tile([P, CH], F32, tag="oaug",
                                       name=f"oaugA_{c}_{p}")
                oaugB = oaug_pool.tile([P, CH], F32, tag="oaug",
                                       name=f"oaugB_{c}_{p}")
                expPs = []
                for m in range(MT):
                    msl = slice(m * P, (m + 1) * P)
                    s_ps = sps_pool.tile([P, 2 * CH], F32, tag="sps",
                                         name=f"sps_{c}_{p}_{m}")
                    # packed S matmuls: head A rows 0:64, head B rows 64:128
                    # (disjoint PE row groups, run concurrently)
                    nc.tensor.matmul(s_ps[:, 0:CH], kTl[0:HD, msl], qA,
                                     start=True, stop=True)
                    nc.tensor.matmul(s_ps[:, CH:2 * CH], kTl[HD:P, msl], qB,
                                     start=True, stop=True)
                    expP = exp_pool.tile([P, 2 * CH], F16, tag="expp",
                                         name=f"expP_{c}_{p}_{m}")
                    nc.scalar.activation(expP[:], s_ps[:],
                                         mybir.ActivationFunctionType.Exp,
                                         scale=SCALE)
                    expPs.append(expP)
                for m in range(MT):
                    nc.tensor.matmul(oaugA[0:HD + 1, :],
                                     v_aug[:, m, 2 * p, :],
                                     expPs[m][:, 0:CH],
                                     start=(m == 0), stop=(m == MT - 1))
                    nc.tensor.matmul(oaugB[0:HD + 1, :],
                                     v_aug[:, m, 2 * p + 1, :],
                                     expPs[m][:, CH:2 * CH],
                                     start=(m == 0), stop=(m == MT - 1))

                # evacuate + reciprocal inline (DVE only; frees oaug fast)
                o_sbs = []
                for half, oaug in ((0, oaugA), (1, oaugB)):
                    o_sb = osb_pool.tile([HD + 1, CH], F32, tag="osb",
                                         name=f"osb_{c}_{p}_{half}")
                    nc.vector.tensor_copy(o_sb[:], oaug[0:HD + 1, :])
                    den = den_pool.tile([HD + 1, CH], F16, tag="den",
                                        name=f"den_{c}_{p}_{half}")
                    nc.vector.tensor_copy(den[HD:HD + 1, :],
                                          o_sb[HD:HD + 1, :])
                    o_sbs.append((o_sb, den))
                return (c, p, o_sbs)

            def emit_norm(unit):
                # broadcast (PE ones-matmul) + normalize; emitted one unit
                # late so the PE reaches the bcast long after recip is done
                c, p, o_sbs = unit
                ot_p = ot_pool.tile([P, CH], F16, name=f"ot_{c}_{p}", tag="ot")
                for half, (o_sb, den) in ((0, o_sbs[0]), (1, o_sbs[1])):
                    bc = aux_pool.tile([HD, CH], F32, tag="aux",
                                       name=f"bc_{c}_{p}_{half}")
                    nc.tensor.matmul(bc[:], ones_r[HD:HD + 1, :],
                                     den[HD:HD + 1, :], start=True, stop=True)
                    rbc = rbc_pool.tile([HD, CH], F32, tag="rbc",
                                        name=f"rbc_{c}_{p}_{half}")
                    with nc.allow_low_precision(reason="softmax denom"):
                        nc.vector.reciprocal(rbc[:], bc[:])
                    nc.vector.tensor_tensor(
                        ot_p[half * HD:(half + 1) * HD, :],
                        o_sb[0:HD, :], rbc[:], mybir.AluOpType.mult)
                ot_map[(c, p)] = ot_p

            def emit_qproj(c, f):
                csl = slice(c * CH, (c + 1) * CH)
                ps = aux_pool.tile([P, CH], F32, tag="aux",
                                   name=f"qp_{c}_{f}")
                for k in range(KT):
                    nc.tensor.matmul(ps[:], wq[f][:, k, :], xt[k][:, csl],
                                     start=(k == 0), stop=(k == KT - 1))
                nc.vector.tensor_copy(qkT[f][:, csl], ps[:])

            def emit_outproj(c):
                csl = slice(c * CH, (c + 1) * CH)
                for e in range(ET):
                    pso = aux_pool.tile([P, CH], F32, tag="aux",
                                        name=f"pso_{c}_{e}")
                    for p in range(NPAIR):
                        nc.tensor.matmul(pso[:],
                                         wo[p][:, e * P:(e + 1) * P],
                                         ot_map[(c, p)][:],
                                         start=(p == 0), stop=(p == NPAIR - 1))
                    st = stage_pool.tile([P, CH], F32, tag="stg",
                                         name=f"st_{c}_{e}")
                    nc.scalar.copy(st[:], pso[:])
                    nc.sync.dma_start(outT_r[e][:, csl], st[:])

            # software pipeline: the norm for unit i is emitted after unit
            # i+1's matmuls, and chunk c's out-projection after chunk c+1's
            # second unit, so the PE never waits on the DVE chain
            from collections import deque
            pend = deque()
            for c in range(NCHUNK):
                for p in range(NPAIR):
                    pend.append(emit_unit(c, p))
                    if len(pend) > 2:
                        emit_norm(pend.popleft())
                    if c + 1 < NCHUNK and p in (0, 1):
                        emit_qproj(c + 1, 2 * p)
                        emit_qproj(c + 1, 2 * p + 1)
                    if p == 2 and c > 0:
                        while pend and pend[0][0] < c:
                            emit_norm(pend.popleft())
                        emit_outproj(c - 1)
            while pend:
                emit_norm(pend.popleft())
            emit_outproj(NCHUNK - 1)

    nc.compile()
    return nc


def _get_nc():
    global _NC_CACHE
    if _NC_CACHE is None:
        _NC_CACHE = _build()
    return _NC_CACHE


def _make_in_maps(x, w_qkv, w_out):
    ones = np.ones((P, HD), dtype=np.float16)
    per_g = []
    for g in range(2):
        qk_g = np.concatenate([w_qkv[g * 512:(g + 1) * 512],
                               w_qkv[DI + g * 512:DI + (g + 1) * 512]], axis=0)
        wqkT = np.ascontiguousarray(qk_g.T)               # [1024 d, 1024 f]
        wqkA = np.ascontiguousarray(
            wqkT.reshape(KT, P, 8, P).transpose(2, 1, 0, 3).astype(np.float16))
        v_g = w_qkv[2 * DI + g * 512:2 * DI + (g + 1) * 512]
        wvT = np.ascontiguousarray(v_g.T)                 # [1024 d, 512 f]
        wvA = np.ascontiguousarray(
            wvT.reshape(KT, P, 512).transpose(1, 0, 2).astype(np.float16))
        woTg = np.ascontiguousarray(
            w_out[:, g * 512:(g + 1) * 512].T.astype(np.float16))
        per_g.append((wqkA, wvA, woTg))

    in_maps = []
    for c in range(8):
        b, g = c // 2, c % 2
        wqkA, wvA, woTg = per_g[g]
        in_maps.append({
            "xT": np.ascontiguousarray(x[b].T.astype(np.float16)),
            "wqkA": wqkA,
            "wvA": wvA,
            "woT": woTg,
            "ones": ones,
        })
    return in_maps


def kernel(x, w_qkv, w_out, b_out):
    x = np.asarray(x, dtype=np.float32)
    w_qkv = np.asarray(w_qkv, dtype=np.float32)
    w_out = np.asarray(w_out, dtype=np.float32)
    b_out = np.asarray(b_out, dtype=np.float32)
    B = x.shape[0]

    in_maps = _make_in_maps(x, w_qkv, w_out)
    nc = _get_nc()
    res = run_bass_kernel_spmd(nc, in_maps, core_ids=list(range(8)))
    parts = [r["outT"] for r in res.results]
    out = np.empty((B, N, DI), dtype=np.float32)
    for b in range(B):
        out[b] = (parts[2 * b] + parts[2 * b + 1]).T + b_out
    return out



# revision 4
# speedup vs baseline: 1.1195x; 1.1195x over previous
"""Multi-head attention Trainium2 kernel (B=4, N=2048, D=1024, H=16).

Sharding: 8 cores = 4 batches x 2 head-groups (8 heads each), zero
collectives. Each core:
  - projections in fp16 (k/v + first q chunk up front; later q chunks are
    projected mid-attention as PE gap filler)
  - q,k kept transposed [feat, seq]; v row-layout, augmented with a ones
    column so the PV matmul emits the softmax denominator for free
  - attention per head-pair x 512-query chunk: S matmuls packed two heads
    per pass via disjoint PE row groups into one [128,1024] PSUM tile,
    one wide exp on ACT (scale=1/8, fp16 out), PV accumulation with
    128-column stationaries (padded windows into the v tile) so the
    weight loads take the fast-load path and hide under matmuls
  - normalization (merged across the head pair): both denominators copied
    to adjacent partitions (fp16), one 2-row broadcast matmul expands them
    across 128 partitions, one reciprocal_approx_fast on the broadcast,
    one tensor_tensor multiply emits the normalized [128, 512] fp16 tile
  - out-projection partial [1024,2048] per chunk, evacuated via DVE
    (ACT stays dedicated to exp), emission deferred past the next chunk's
    start so the PE queue never blocks on the DVE chain
Host sums the two head-group partials per batch and adds bias.
"""
from contextlib import ExitStack

import numpy as np

import concourse.mybir as mybir
import concourse.tile as tile
from concourse import bacc
from concourse.bass_utils import run_bass_kernel_spmd

F32 = mybir.dt.float32
F16 = mybir.dt.float16

P = 128
N = 2048         # sequence length
DI = 1024        # model dim
NH = 8           # heads per core
HD = 64          # head dim
NPAIR = 4        # head pairs per core
KT = 8           # contraction tiles for projections
CH = 512         # query chunk width
NCHUNK = 4       # chunks per sequence
MT = 16          # key tiles (m) per sequence
ET = 8           # output-feature blocks
SCALE = HD ** -0.5
VW = HD + 1      # v columns per head incl. denominator ones-column
VFLAT = MT * NH * VW

_NC_CACHE = None


def _build():
    nc = bacc.Bacc("TRN2", target_bir_lowering=False, debug=False)

    xT = nc.dram_tensor("xT", [DI, N], F16, kind="ExternalInput").ap()
    wqkA = nc.dram_tensor("wqkA", [8, P, KT, P], F16, kind="ExternalInput").ap()
    wvA = nc.dram_tensor("wvA", [P, KT, 512], F16, kind="ExternalInput").ap()
    woT = nc.dram_tensor("woT", [512, DI], F16, kind="ExternalInput").ap()
    cstd = nc.dram_tensor("cst", [P, 129], F16, kind="ExternalInput").ap()
    outT = nc.dram_tensor("outT", [DI, N], F32, kind="ExternalOutput").ap()

    xT_r = xT.rearrange("(k p) n -> k p n", p=P)        # [8, 128, 2048]
    woT_r = woT.rearrange("(k p) e -> k p e", p=P)      # [4, 128, 1024]
    outT_r = outT.rearrange("(e p) n -> e p n", p=P)    # [8, 128, 2048]

    with tile.TileContext(nc) as tc, ExitStack() as persist:
        qk_pool = persist.enter_context(tc.tile_pool(name="qkp", bufs=8))
        va_pool = persist.enter_context(tc.tile_pool(name="vap", bufs=1))
        misc = persist.enter_context(tc.tile_pool(name="misc", bufs=1))
        wqk_pool = persist.enter_context(tc.tile_pool(name="wqk", bufs=2))
        xt_pool = persist.enter_context(tc.tile_pool(name="xt", bufs=8))
        wv_pool = persist.enter_context(tc.tile_pool(name="wv", bufs=1))
        wq_pool = persist.enter_context(tc.tile_pool(name="wq", bufs=4))

        # DMA order matters: the very first matmul group (k-projection,
        # feature block 4) needs wqk[4] + xt[0] — issue those first so the
        # PE starts ~3us in instead of waiting for the full 4MB x load.
        cst = misc.tile([P, 129], F16)
        nc.sync.dma_start(cst[:], cstd[:])
        wqk_first = wqk_pool.tile([P, KT, P], F16, tag="wqk")
        nc.sync.dma_start(wqk_first[:], wqkA[4])
        xt = []
        for k in range(KT):
            t = xt_pool.tile([P, N], F16, name=f"xt{k}", tag="xt")
            nc.sync.dma_start(t[:], xT_r[k])
            xt.append(t)
        wv = wv_pool.tile([P, KT, 512], F16)
        nc.sync.dma_start(wv[:], wvA[:])
        wq = []
        for f in range(4):
            t = wq_pool.tile([P, KT, P], F16, name=f"wq{f}", tag="wq")
            nc.sync.dma_start(t[:], wqkA[f])
            wq.append(t)

        # qkT tiles: 0..3 = q head-pairs, 4..7 = k head-pairs.
        # Tile j holds heads 2j (parts 0:64) and 2j+1 (parts 64:128).
        qkT = [qk_pool.tile([P, N], F16, name=f"qkT{t}", tag="qkT")
               for t in range(8)]
        # v tile: flat [P, VFLAT+64]; logical view [P, MT, NH, VW] with 64
        # elements of slack so every per-(m,h) stationary window can be
        # read 128 columns wide (full-width weights take the fast-load
        # path and hide under in-flight matmuls).
        va_t = va_pool.tile([P, VFLAT + 64], F16)
        nc.vector.memset(va_t[:, VFLAT:VFLAT + 64], 0.0)
        v_aug = va_t[:, 0:VFLAT].rearrange("p (m h d) -> p m h d", h=NH, d=VW)
        nc.vector.tensor_copy(v_aug[:, :, :, HD:HD + 1],
                              cst[:, 0:1].to_broadcast((P, MT, NH, 1)))

        # ---------------- Phase 1: k/v projections + first q chunk --------
        with ExitStack() as ph1:
            pp = ph1.enter_context(tc.tile_pool(name="pp", bufs=4, space="PSUM"))

            # k projection: feature blocks 4..7, all chunks
            for f in range(4, 8):
                if f == 4:
                    wqk_f = wqk_first
                else:
                    wqk_f = wqk_pool.tile([P, KT, P], F16, tag="wqk")
                    nc.sync.dma_start(wqk_f[:], wqkA[f])
                for c in range(NCHUNK):
                    ps = pp.tile([P, CH], F32, tag="pp")
                    for k in range(KT):
                        nc.tensor.matmul(ps[:], wqk_f[:, k, :],
                                         xt[k][:, c * CH:(c + 1) * CH],
                                         start=(k == 0), stop=(k == KT - 1))
                    nc.vector.tensor_copy(qkT[f][:, c * CH:(c + 1) * CH], ps[:])

            # v projection: row block r
            for r in range(MT):
                ps = pp.tile([P, CH], F32, tag="pp")
                for k in range(KT):
                    nc.tensor.matmul(ps[:], xt[k][:, r * P:(r + 1) * P],
                                     wv[:, k, :],
                                     start=(k == 0), stop=(k == KT - 1))
                nc.vector.tensor_copy(v_aug[:, r, :, 0:HD],
                                      ps.rearrange("p (h d) -> p h d", d=HD))

            # q projection for chunk 0 only
            for f in range(4):
                ps = pp.tile([P, CH], F32, tag="pp")
                for k in range(KT):
                    nc.tensor.matmul(ps[:], wq[f][:, k, :], xt[k][:, 0:CH],
                                     start=(k == 0), stop=(k == KT - 1))
                nc.vector.tensor_copy(qkT[f][:, 0:CH], ps[:])

        # ---------------- Phase 2: attention + out-projection ----------------
        with ExitStack() as ph2:
            wo_pool = ph2.enter_context(tc.tile_pool(name="wo", bufs=4))
            exp_pool = ph2.enter_context(tc.tile_pool(name="expp", bufs=18))
            ot_pool = ph2.enter_context(tc.tile_pool(name="ot", bufs=8))
            osb_pool = ph2.enter_context(tc.tile_pool(name="osb", bufs=4))
            stage_pool = ph2.enter_context(tc.tile_pool(name="stg", bufs=3))
            den_pool = ph2.enter_context(tc.tile_pool(name="den", bufs=4))
            rbc_pool = ph2.enter_context(tc.tile_pool(name="rbc", bufs=2))
            sps_pool = ph2.enter_context(
                tc.tile_pool(name="sps", bufs=2, space="PSUM"))
            oaug_pool = ph2.enter_context(
                tc.tile_pool(name="oaug", bufs=2, space="PSUM"))
            aux_pool = ph2.enter_context(
                tc.tile_pool(name="aux", bufs=2, space="PSUM"))

            wo = [wo_pool.tile([P, DI], F16, name=f"wo{kk}", tag="wo")
                  for kk in range(NPAIR)]
            for kk in range(NPAIR):
                nc.sync.dma_start(wo[kk][:], woT_r[kk])

            ot_map = {}

            def emit_unit(c, p):
                csl = slice(c * CH, (c + 1) * CH)
                qA = qkT[p][0:HD, csl]
                qB = qkT[p][HD:P, csl]
                kTl = qkT[4 + p]

                oaugA = oaug_pool.tile([P, CH], F32, tag="oaug",
                                       name=f"oaugA_{c}_{p}")
                oaugB = oaug_pool.tile([P, CH], F32, tag="oaug",
                                       name=f"oaugB_{c}_{p}")
                expPs = []
                for m in range(MT):
                    msl = slice(m * P, (m + 1) * P)
                    s_ps = sps_pool.tile([P, 2 * CH], F32, tag="sps",
                                         name=f"sps_{c}_{p}_{m}")
                    # packed S matmuls: head A rows 0:64, head B rows 64:128
                    # (disjoint PE row groups, run concurrently)
                    nc.tensor.matmul(s_ps[:, 0:CH], kTl[0:HD, msl], qA,
                                     start=True, stop=True)
                    nc.tensor.matmul(s_ps[:, CH:2 * CH], kTl[HD:P, msl], qB,
                                     start=True, stop=True)
                    expP = exp_pool.tile([P, 2 * CH], F16, tag="expp",
                                         name=f"expP_{c}_{p}_{m}")
                    nc.scalar.activation(expP[:], s_ps[:],
                                         mybir.ActivationFunctionType.Exp,
                                         scale=SCALE)
                    expPs.append(expP)
                for m in range(MT):
                    vbase = (m * NH + 2 * p) * VW
                    nc.tensor.matmul(oaugA[:, :],
                                     va_t[:, vbase:vbase + P],
                                     expPs[m][:, 0:CH],
                                     start=(m == 0), stop=(m == MT - 1))
                    vbase = (m * NH + 2 * p + 1) * VW
                    nc.tensor.matmul(oaugB[:, :],
                                     va_t[:, vbase:vbase + P],
                                     expPs[m][:, CH:2 * CH],
                                     start=(m == 0), stop=(m == MT - 1))

                # evacuate numerators + both denominators (DVE only)
                o_sb = osb_pool.tile([P, CH], F32, tag="osb",
                                     name=f"osb_{c}_{p}")
                denA = den_pool.tile([1, CH], F16, tag="den",
                                     name=f"denA_{c}_{p}")
                denB = den_pool.tile([1, CH], F16, tag="den",
                                     name=f"denB_{c}_{p}")
                nc.vector.tensor_copy(o_sb[0:HD, :], oaugA[0:HD, :])
                nc.vector.tensor_copy(o_sb[HD:P, :], oaugB[0:HD, :])
                with nc.allow_low_precision(reason="softmax denom fp16"):
                    nc.vector.tensor_copy(denA[:], oaugA[HD:HD + 1, :])
                    nc.vector.tensor_copy(denB[:], oaugB[HD:HD + 1, :])
                return (c, p, o_sb, denA, denB)

            def emit_norm(unit):
                # broadcast (2-row PE matmul) + reciprocal + multiply;
                # emitted one unit late so the PE reaches the bcast long
                # after the denominators land
                c, p, o_sb, denA, denB = unit
                bc = aux_pool.tile([P, CH], F32, tag="aux",
                                   name=f"bc_{c}_{p}")
                nc.tensor.matmul(bc[0:HD, :], cst[0:1, 1:65], denA[:],
                                 start=True, stop=True)
                nc.tensor.matmul(bc[HD:P, :], cst[0:1, 1:65], denB[:],
                                 start=True, stop=True)
                rbc = rbc_pool.tile([P, CH], F32, tag="rbc",
                                    name=f"rbc_{c}_{p}")
                nc.vector.reciprocal_approx_fast(out=rbc[:], in_=bc[:])
                ot_p = ot_pool.tile([P, CH], F16, name=f"ot_{c}_{p}", tag="ot")
                nc.vector.tensor_tensor(ot_p[:], o_sb[:], rbc[:],
                                        mybir.AluOpType.mult)
                ot_map[(c, p)] = ot_p

            def emit_qproj(c, f):
                csl = slice(c * CH, (c + 1) * CH)
                ps = aux_pool.tile([P, CH], F32, tag="aux",
                                   name=f"qp_{c}_{f}")
                for k in range(KT):
                    nc.tensor.matmul(ps[:], wq[f][:, k, :], xt[k][:, csl],
                                     start=(k == 0), stop=(k == KT - 1))
                nc.vector.tensor_copy(qkT[f][:, csl], ps[:])

            def emit_outproj(c):
                csl = slice(c * CH, (c + 1) * CH)
                for e in range(ET):
                    pso = aux_pool.tile([P, CH], F32, tag="aux",
                                        name=f"pso_{c}_{e}")
                    for p in range(NPAIR):
                        nc.tensor.matmul(pso[:],
                                         wo[p][:, e * P:(e + 1) * P],
                                         ot_map[(c, p)][:],
                                         start=(p == 0), stop=(p == NPAIR - 1))
                    st = stage_pool.tile([P, CH], F32, tag="stg",
                                         name=f"st_{c}_{e}")
                    nc.vector.tensor_copy(st[:], pso[:])
                    nc.sync.dma_start(outT_r[e][:, csl], st[:])

            # software pipeline: the norm for unit i is emitted after unit
            # i+1's matmuls, and chunk c's out-projection after chunk c+1's
            # second unit, so the PE never waits on the DVE chain
            from collections import deque
            pend = deque()
            for c in range(NCHUNK):
                for p in range(NPAIR):
                    pend.append(emit_unit(c, p))
                    if len(pend) > 2:
                        emit_norm(pend.popleft())
                    if c + 1 < NCHUNK and p in (0, 1):
                        emit_qproj(c + 1, 2 * p)
                        emit_qproj(c + 1, 2 * p + 1)
                    if p == 2 and c > 0:
                        while pend and pend[0][0] < c:
                            emit_norm(pend.popleft())
                        emit_outproj(c - 1)
            while pend:
                emit_norm(pend.popleft())
            emit_outproj(NCHUNK - 1)

    nc.compile()
    return nc


def _get_nc():
    global _NC_CACHE
    if _NC_CACHE is None:
        _NC_CACHE = _build()
    return _NC_CACHE


def _make_in_maps(x, w_qkv, w_out):
    cst = np.zeros((P, 129), dtype=np.float16)
    cst[:, 0] = 1.0
    cst[0, 1:65] = 1.0
    cst[1, 65:129] = 1.0
    per_g = []
    for g in range(2):
        qk_g = np.concatenate([w_qkv[g * 512:(g + 1) * 512],
                               w_qkv[DI + g * 512:DI + (g + 1) * 512]], axis=0)
        wqkT = np.ascontiguousarray(qk_g.T)               # [1024 d, 1024 f]
        wqkA = np.ascontiguousarray(
            wqkT.reshape(KT, P, 8, P).transpose(2, 1, 0, 3).astype(np.float16))
        v_g = w_qkv[2 * DI + g * 512:2 * DI + (g + 1) * 512]
        wvT = np.ascontiguousarray(v_g.T)                 # [1024 d, 512 f]
        wvA = np.ascontiguousarray(
            wvT.reshape(KT, P, 512).transpose(1, 0, 2).astype(np.float16))
        woTg = np.ascontiguousarray(
            w_out[:, g * 512:(g + 1) * 512].T.astype(np.float16))
        per_g.append((wqkA, wvA, woTg))

    in_maps = []
    for c in range(8):
        b, g = c // 2, c % 2
        wqkA, wvA, woTg = per_g[g]
        in_maps.append({
            "xT": np.ascontiguousarray(x[b].T.astype(np.float16)),
            "wqkA": wqkA,
            "wvA": wvA,
            "woT": woTg,
            "cst": cst,
        })
    return in_maps


def kernel(x, w_qkv, w_out, b_out):
    x = np.asarray(x, dtype=np.float32)
    w_qkv = np.asarray(w_qkv, dtype=np.float32)
    w_out = np.asarray(w_out, dtype=np.float32)
    b_out = np.asarray(b_out, dtype=np.float32)
    B = x.shape[0]

    in_maps = _make_in_maps(x, w_qkv, w_out)
    nc = _get_nc()
    res = run_bass_kernel_spmd(nc, in_maps, core_ids=list(range(8)))
    parts = [r["outT"] for r in res.results]
    out = np.empty((B, N, DI), dtype=np.float32)
    for b in range(B):
        out[b] = (parts[2 * b] + parts[2 * b + 1]).T + b_out
    return out
